# revision 42
# baseline (speedup 1.0000x reference)
"""Trainium2 Bass kernel for the dense_cnn problem.

Computes out = (x + conv(x)) * t4 where
  conv = Conv2d(64->64, kernel (1,7), dilation (1,3), padding (0,9), no bias)
  t4[n,c,h,w] = sum_k p4w[k] * unfold3_dil2_h(x) rolled by (+1 h, -2 w)
             = roll_w(-2)[ p0*x[h-3] + p1*x[h-1] + p2*x[h+1] ]   (h taps via
               g=(h-1)%128; rows outside [0,128) contribute zero)

Sharding: pure data parallel, batch 32 -> 8 cores x 4 items. Each core
processes its 4 items as 2 "pairs": two batch items stacked on the 128
SBUF partitions (partition = 64*b + c).

Streamed over 32-row superblocks (descending h so edge rows for h in
{0,1,2} can read the tail rows captured into a small side tile), with the
two pairs' superblocks interleaved (96p0, 96p1, 64p0, ...) so every
cross-stage dependency has a full superblock of slack:
  - PE: warm-up dummy matmuls ramp the p-state; then per 4-row PSUM
    block, 7 block-diagonal f32r conv-tap matmuls on width-shifted chunk
    views (center tap first, start=True, carrying the residual identity;
    edge taps skip the zero-pad output columns); weights pre-scaled by sm.
  - ACT: u = sa*x[h+o0] (copy-scale, in row-quarters); issues the
    previous iteration's two half-stores lag-one so their sem park never
    delays the chain head.
  - GPSIMD/Pool: u += x[h+oj] (in-place add, quarters); chunk pad
    memsets; 4-row halo copy from the previous chunk (saves re-reading
    the halo from HBM).
  - DVE: v = sc*x[h+o2] + u (quarters), built pre-rolled in w (the
    circular roll is applied at this combine); final out = psum * v per
    4-row block, one tensor_tensor multiply, PSUM read direct. The last
    superblock's stores go out in fine pieces from SP to shrink the tail.
"""

import sys

for _p in ("/opt/trn_rl_repo", "/opt/trn_rl_repo/concourse"):
    if _p not in sys.path:
        sys.path.insert(0, _p)

import numpy as np

N, C, H, W = 32, 64, 128, 128
N_CORES = 8
N_PER_CORE = N // N_CORES          # 4
PAIRS_PER_CORE = N_PER_CORE // 2   # 2
SB = 32                            # superblock rows
HALO_LO, HALO_HI = 3, 1            # x rows [s-3, s+33) needed per superblock
CHUNK_ROWS = SB + HALO_LO + HALO_HI  # 36
WP = W + 18                        # padded row stride for conv taps (9 each side)
TAP_OFFS = (-3, -1, 1)             # x-row offset of t4 tap k (bulk rows h>=3, h<=126)
CONV_D = tuple(3 * t - 9 for t in range(7))  # width offsets of the 7 conv taps

_CACHE = {}


def _special_terms(h):
    """(coeff_index, x_row) terms of t4 row h that fall inside [0, H)."""
    g = (h - 1) % H
    out = []
    for k in range(3):
        r = g + 2 * (k - 1)
        if 0 <= r < H:
            out.append((k, r))
    return out


def _build_bass(p):
    """Build the per-core Bass program. p = the 3 t4 tap coefficients."""
    import concourse.bass as bass
    import concourse.bacc as bacc
    import concourse.mybir as mybir
    import concourse.tile as tile

    dt = mybir.dt
    AL = mybir.AluOpType

    j = int(np.argmax(np.abs(p)))
    o0, o2 = [k for k in range(3) if k != j]
    sa = float(p[o0] / p[j])
    sc = float(p[o2] / p[j])
    sm = float(p[j])

    f32 = dt.float32
    f32r = dt.float32r

    nc = bacc.Bacc()
    x_d = nc.dram_tensor("x", [N_PER_CORE * C, H * W], f32r, kind="ExternalInput")
    w_d = nc.dram_tensor("wts", [128, 7 * 128], f32r, kind="ExternalInput")
    o_d = nc.dram_tensor("out", [N_PER_CORE * C, H * W], f32, kind="ExternalOutput")

    with tile.TileContext(nc) as tc:
        with (
            tc.tile_pool(name="wpool", bufs=1) as wpool,
            tc.tile_pool(name="chunk", bufs=4) as chp,
            tc.tile_pool(name="upool", bufs=2) as upool,
            tc.tile_pool(name="vpool", bufs=2) as vpool,
            tc.tile_pool(name="opool", bufs=2) as opool,
            tc.tile_pool(name="side", bufs=2) as sidep,
            tc.tile_pool(name="psum", bufs=8, space="PSUM") as psp,
        ):
            wt = wpool.tile([128, 7 * 128], f32r)
            nc.sync.dma_start(wt[:], w_d[:, :])

            # p-state warm-up: dependency-free dummy matmuls keep the PE busy
            # through the startup window so the real matmuls are issued (and
            # cost-stamped) at the ramped clock instead of the cold one
            scr = wpool.tile([128, 256], f32r)
            nc.gpsimd.memset(scr[:].bitcast(f32), 0.0)
            warm = psp.tile([128, 4 * W], f32, name="ps", tag="ps")
            for _ in range(18):
                nc.tensor.matmul(
                    warm[:, 0:256], scr[:, 0:128], scr[:], start=True, stop=True
                )

            # Superblocks of the two batch pairs are interleaved (96p0, 96p1,
            # 64p0, ...) so every cross-stage dependency (v -> finals -> PSUM
            # bank reuse, chunk buffer recycling, store drains) has a full
            # superblock of slack: while one pair's superblock runs on the
            # PE, the other pair's t4/conversion chain fills.
            pend_out = []  # lag-one stores, flushed at the next u-point
            prev_chfs = {}
            side3s = {}
            for s in (96, 64, 32, 0):
                for pair in range(PAIRS_PER_CORE):
                    rows = slice(pair * 128, (pair + 1) * 128)
                    if s == 96:
                        side = sidep.tile([128, 4 * W], f32)  # x rows 124..127
                        side3s[pair] = side[:].rearrange("p (h w) -> p h w", w=W)
                    side3 = side3s[pair]
                    prev_chf = prev_chfs.get(pair)
                    ch = chp.tile([128, CHUNK_ROWS * WP], f32r)
                    chp3 = ch[:].rearrange("p (h w) -> p h w", w=WP)
                    # zero the 9-col pads once per chunk (cheap, strided);
                    # on gpsimd to keep the DVE queue free for finals
                    chpf = ch[:].bitcast(f32).rearrange("p (h w) -> p h w", w=WP)
                    nc.gpsimd.memset(chpf[:, :, 0:9], 0.0)
                    nc.gpsimd.memset(chpf[:, :, 9 + W : WP], 0.0)
                    # chunk row r  <->  x row (s - HALO_LO) + r
                    chf = chpf[:, :, 9 : 9 + W]
                    chr = lambda xr: xr - (s - HALO_LO)  # x row -> chunk row
                    if s == 96:
                        # first superblock of the pair: full 35-row load,
                        # in pieces so the first blocks can start sooner
                        pieces = (0, 8, 14, 20, 35) if pair == 0 else (0, 8, 20, 35)
                        for a, b in zip(pieces, pieces[1:]):
                            nc.sync.dma_start(
                                chp3[:, a:b, 9 : 9 + W],
                                x_d[rows, (93 + a) * W : (93 + b) * W],
                            )
                    else:
                        # new rows from HBM; the 4 halo rows [s+29, s+33)
                        # are copied from the previous (s+32) chunk instead
                        # of re-read (they are its rows 0..4)
                        lo = max(0, s - 3)
                        nc.sync.dma_start(
                            chp3[:, chr(lo) : 32, 9 : 9 + W],
                            x_d[rows, lo * W : (s + 29) * W],
                        )
                        # copy via the f32r views: the PE consumes these rows
                        # and the verifier requires an f32r-typed producer
                        nc.gpsimd.tensor_copy(
                            chp3[:, 32:36, 9 : 9 + W], prev_chf[:, 0:4, 9 : 9 + W]
                        )
                    prev_chfs[pair] = chp3
                    ch3 = chp3[:, :, :]  # f32r, PE
                    if s == 96:
                        nc.gpsimd.tensor_copy(
                            side3[:, :, :], chf[:, chr(124) : chr(128), :]
                        )

                    # ---- t4 bulk: U on ACT+gpsimd, V on DVE ----
                    hlo = max(s, 3)
                    hhi = min(s + SB, 127)  # h=127 handled as a special
                    u = upool.tile([128, SB * W], f32)
                    v = vpool.tile([128, SB * W], f32)
                    u3 = u[:].rearrange("p (h w) -> p h w", w=W)
                    v3 = v[:].rearrange("p (h w) -> p h w", w=W)
                    bs = slice(hlo - s, hhi - s)  # tile-row range of the bulk

                    def cx(off, c0=0, c1=W):
                        r0_ = hlo + off - (s - HALO_LO)
                        r1_ = hhi + off - (s - HALO_LO)
                        return chf[:, r0_:r1_, c0:c1]

                    # Pool has no STT: scale on ACT, add on GPSIMD (in-place).
                    # u is built unrolled; the circular w-roll (-2) is applied
                    # when v is combined, so the final multiply needs no fixup.
                    # u/v are built in row-quarters so the serial chain
                    # ACT(scale) -> Pool(add) -> DVE(combine) pipelines: its
                    # latency (~10us) then fits inside one iteration period.
                    nbulk = hhi - hlo
                    nq = 8 if s == 96 else 4  # eighths while the pipe fills
                    qb = [(nbulk * i) // nq for i in range(nq + 1)]
                    quarters = list(zip(qb, qb[1:]))
                    first_q = True
                    for rb0, rb1 in quarters:
                        bh = slice(bs.start + rb0, bs.start + rb1)

                        def cxh(off, c0=0, c1=W):
                            r0_ = hlo + rb0 + off - (s - HALO_LO)
                            r1_ = hlo + rb1 + off - (s - HALO_LO)
                            return chf[:, r0_:r1_, c0:c1]

                        nc.scalar.activation(
                            u3[:, bh, :], cxh(TAP_OFFS[o0]),
                            mybir.ActivationFunctionType.Copy, scale=sa,
                        )
                        if first_q:
                            # previous iteration's stores, issued on ACT
                            # *after* the first u quarter: their park on
                            # finals never delays the chain head
                            for po in pend_out:
                                nc.scalar.dma_start(*po)
                            pend_out = []
                            first_q = False
                        nc.gpsimd.tensor_add(
                            u3[:, bh, :], u3[:, bh, :], cxh(TAP_OFFS[j])
                        )
                        nc.vector.scalar_tensor_tensor(
                            v3[:, bh, 0 : W - 2], cxh(TAP_OFFS[o2], 2, W), sc,
                            u3[:, bh, 2:W], op0=AL.mult, op1=AL.add,
                        )
                    nc.vector.scalar_tensor_tensor(
                        v3[:, bs, W - 2 : W], cx(TAP_OFFS[o2], 0, 2), sc,
                        u3[:, bs, 0:2], op0=AL.mult, op1=AL.add,
                    )

                    # ---- special t4 rows (unfold zero-pad x roll wrap) ----
                    specials = []
                    if s == 96:
                        specials = [127]
                    elif s == 0:
                        specials = [0, 1, 2]
                    for h in specials:
                        (ka, ra), (kb, rb) = _special_terms(h)
                        if abs(p[ka]) > abs(p[kb]):
                            (ka, ra), (kb, rb) = (kb, rb), (ka, ra)

                        def srcrow(r, c0=0, c1=W):
                            if s == 0 and r >= 124:
                                return side3[:, r - 124 : r - 123, c0:c1]
                            return chf[:, chr(r) : chr(r) + 1, c0:c1]

                        vrow = v3[:, h - s : h - s + 1, :]
                        ratio = float(p[ka] / p[kb])
                        nc.vector.scalar_tensor_tensor(
                            vrow[:, :, 0 : W - 2], srcrow(ra, 2, W), ratio,
                            srcrow(rb, 2, W), op0=AL.mult, op1=AL.add,
                        )
                        nc.vector.scalar_tensor_tensor(
                            vrow[:, :, W - 2 : W], srcrow(ra, 0, 2), ratio,
                            srcrow(rb, 0, 2), op0=AL.mult, op1=AL.add,
                        )
                        nc.vector.tensor_scalar_mul(vrow, vrow, float(p[kb] / sm))

                    # ---- conv + residual on PE, final multiply on DVE ----
                    ot = opool.tile([128, SB * W], f32)
                    o3 = ot[:].rearrange("p (h w) -> p h w", w=W)
                    pss = [
                        psp.tile([128, 4 * W], f32, name="ps", tag="ps")
                        for _ in range(SB // 4)
                    ]
                    for jb in range(SB // 4):
                        hb = s + 4 * jb
                        ps = pss[jb]
                        ps3 = ps[:].rearrange("p (h w) -> p h w", w=W)
                        rh = slice(chr(hb), chr(hb) + 4)
                        # 7 conv taps; the center tap (t=3, d=0) carries the
                        # residual identity and all taps are pre-scaled by sm
                        # on the host, so ps = sm*(x + conv(x)). The center
                        # tap goes first (start=True over the full width);
                        # edge taps skip the |d| output columns whose input
                        # is zero padding, trimming the PE stream.
                        order = (3, 0, 1, 2, 4, 5, 6)
                        for ti, t in enumerate(order):
                            d = CONV_D[t]
                            # even-aligned for the fp32r dst pattern check;
                            # the extra column reads zero padding
                            w0 = (max(0, -d) // 2) * 2
                            w1 = ((min(W, W - d) + 1) // 2) * 2
                            nc.tensor.matmul(
                                ps3[:, :, w0:w1],
                                wt[:, t * 128 : (t + 1) * 128],
                                ch3[:, rh, 9 + d + w0 : 9 + d + w1],
                                start=(ti == 0), stop=(ti == 6),
                            )
                        tr = slice(4 * jb, 4 * jb + 4)
                        # v holds roll_w(-2)(t4)/sm, so out = ps * v directly
                        nc.vector.tensor_mul(o3[:, tr, :], ps3[:, :, :], v3[:, tr, :])
                        if pair == PAIRS_PER_CORE - 1 and s == 0 and (
                            jb in (1, 3) or jb >= 4
                        ):
                            # tail: store the very last superblock from SP
                            # (idle by now) in ever finer pieces as each
                            # becomes ready, so the final drain is minimal
                            h0, hn = (jb - 1) * 4, 8
                            if jb >= 4:
                                h0, hn = jb * 4, 4
                            nc.sync.dma_start(
                                o_d[rows, (s + h0) * W : (s + h0 + hn) * W],
                                ot[:, h0 * W : (h0 + hn) * W],
                            )
                    if not (pair == PAIRS_PER_CORE - 1 and s == 0):
                        half = SB // 2
                        pend_out = [
                            (o_d[rows, s * W : (s + half) * W], ot[:, 0 : half * W]),
                            (o_d[rows, (s + half) * W : (s + SB) * W], ot[:, half * W :]),
                        ]
            for po in pend_out:
                nc.scalar.dma_start(*po)
    nc.compile()
    return nc


def kernel(x, W_conv, p4w):
    x = np.ascontiguousarray(x, dtype=np.float32)
    W_conv = np.asarray(W_conv, dtype=np.float32)
    p = np.asarray(p4w, dtype=np.float64).reshape(3)

    from concourse.bass_utils import run_bass_kernel_spmd

    key = tuple(np.round(p, 12))
    if key not in _CACHE:
        _CACHE[key] = _build_bass(p)
    nc = _CACHE[key]

    # weights: 7 block-diag conv taps, lhsT layout (K=128, M=128), pre-scaled
    # by sm = p[j]; the center tap (w-offset 0) carries the residual identity
    j = int(np.argmax(np.abs(p)))
    sm = float(p[j])
    wts = np.zeros((128, 7 * 128), dtype=np.float32)
    wk = np.asarray(W_conv, dtype=np.float64)[:, :, 0, :]  # (O, I, T)
    for t in range(7):
        blk = wk[:, :, t].T  # (I, O) = lhsT block
        if t == 3:
            blk = blk + np.eye(64)
        blk = (sm * blk).astype(np.float32)
        wts[0:64, t * 128 + 0 : t * 128 + 64] = blk
        wts[64:128, t * 128 + 64 : t * 128 + 128] = blk

    xs = x.reshape(N_CORES, N_PER_CORE * C, H * W)
    in_maps = [{"x": xs[k], "wts": wts} for k in range(N_CORES)]
    res = run_bass_kernel_spmd(nc, in_maps, core_ids=list(range(N_CORES)))
    out = np.stack([res.results[k]["out"] for k in range(N_CORES)])
    return out.reshape(N, C, H, W)
